# revision 1
# baseline (speedup 1.0000x reference)
"""Multi-head attention (B=2, S=2048, D=1024, H=16) on 8 Trainium2 NeuronCores.

Sharding: core c handles batch b = c//4 and head-group g = c%4 (4 heads,
a 256-wide column slice of wq/wk/wv and row slice of wo).  Each core
computes a full [S, D] partial of the output projection; the host sums
the 4 partials per batch and adds the output bias.

Per-core kernel (all layouts chosen so softmax runs over the PSUM
partition axis and every DVE/ACT op stays partition-aligned):
  - QT/KT = (x @ w)^T in [head_dim, S] layout via fp32r matmuls
    (activations pre-transposed on host, weights as-is).
  - VH = x @ wv in natural [S, head_cols] layout, bf16, with a ones
    column appended per head (yields softmax denominators for free).
  - scores^T per head-pair via row-tiled (tile_position) K=64 matmuls,
    exp on the scalar engine (scale=1/8 folded in), probs in bf16.
  - ctx^T = VH_aug^T @ probs^T accumulated over S chunks; row 64 of the
    PSUM tile is the softmax denominator.
  - normalize with vector-engine reciprocal + a K=1 replicate matmul.
  - out partial = ctx^T.T @ wo_slice in fp32r.
"""

import os
import sys

import ml_dtypes
import numpy as np

if "/opt/trn_rl_repo" not in sys.path:
    sys.path.insert(0, "/opt/trn_rl_repo")

B, S, D, H = 2, 2048, 1024, 16
DH = D // H  # 64
NCORES = 8
GC = 256  # column slice per core (4 heads)
NP = 2  # head pairs per core
KC = D // 128  # 8 contraction chunks
SQC = S // 512  # 4 query chunks
SKC = S // 128  # 16 key chunks

_CACHE = {}


def _build_program():
    import concourse.bass as bass
    import concourse.tile as tile
    from concourse import bacc, mybir

    F32 = mybir.dt.float32
    F32R = mybir.dt.float32r
    BF16 = mybir.dt.bfloat16
    EXP = mybir.ActivationFunctionType.Exp
    PSUM = bass.MemorySpace.PSUM

    nc = bacc.Bacc()

    qT = nc.dram_tensor("qT", (D, S), BF16, kind="ExternalInput").ap()
    kT = nc.dram_tensor("kT", (D, S), BF16, kind="ExternalInput").ap()
    vT = nc.dram_tensor("vT", (D, S), BF16, kind="ExternalInput").ap()
    wqs = nc.dram_tensor("wqs", (D, GC), BF16, kind="ExternalInput").ap()
    wks = nc.dram_tensor("wks", (D, GC), BF16, kind="ExternalInput").ap()
    wvs = nc.dram_tensor("wvs", (D, GC), BF16, kind="ExternalInput").ap()
    wos = nc.dram_tensor("wos", (GC, D), BF16, kind="ExternalInput").ap()
    bqs = nc.dram_tensor("bqs", (NP, 128, 1), F32, kind="ExternalInput").ap()
    bks = nc.dram_tensor("bks", (NP, 128, 1), F32, kind="ExternalInput").ap()
    bvs = nc.dram_tensor("bvs", (1, GC), F32R, kind="ExternalInput").ap()
    outp = nc.dram_tensor("outp", (S, D), F32, kind="ExternalOutput").ap()

    with tile.TileContext(nc) as tc:
        with (
            tc.tile_pool(name="const", bufs=1) as const,
            tc.tile_pool(name="raw", bufs=1) as rawp,
            tc.tile_pool(name="probs", bufs=10) as probs,
            tc.tile_pool(name="small", bufs=2) as small,
            tc.tile_pool(name="outsb", bufs=3) as outsb,
            tc.tile_pool(name="pproj", bufs=2, space=PSUM) as pproj,
            tc.tile_pool(name="psc", bufs=2, space=PSUM) as psc,
            tc.tile_pool(name="pctx", bufs=1, space=PSUM) as pctx,
        ):
            # ---- constants / weights ----
            # wq/wk/wv as one [128, KC, GC] bf16 tile each (single DMA)
            wq_t = const.tile([128, KC, GC], BF16, name="wqt", tag="wqt")
            wk_t = const.tile([128, KC, GC], BF16, name="wkt", tag="wkt")
            wv_t = const.tile([128, KC, GC], BF16, name="wvt", tag="wvt")
            wo_t = [const.tile([128, D], BF16, name=f"wo{m}", tag=f"wo{m}") for m in range(NP)]
            bq_t = [const.tile([128, 1], F32, name=f"bq{m}", tag=f"bq{m}") for m in range(NP)]
            bk_t = [const.tile([128, 1], F32, name=f"bk{m}", tag=f"bk{m}") for m in range(NP)]
            bv_row = const.tile([1, GC], F32R, name="bvrow", tag="bvrow")
            ones_t = const.tile([128, 128], F32R, name="ones", tag="ones")

            # PE warmup chain: keeps the HAM clock-gate at 8/8 through the
            # initial input DMA window so projections start at 2.4 GHz.
            wu = const.tile([128, 512], BF16, name="wu", tag="wu")
            nc.vector.memset(wu[:], 0.0)
            wup = psc.tile([128, 1024], F32, name="sc", tag="sc")
            for w in range(70):
                nc.tensor.matmul(
                    wup[:, 0:512], wu[:, 0:128], wu[:],
                    start=(w == 0), stop=(w == 69),
                )

            nc.sync.dma_start(wq_t[:], wqs.rearrange("(c p) g -> p c g", p=128))
            nc.sync.dma_start(wk_t[:], wks.rearrange("(c p) g -> p c g", p=128))
            nc.sync.dma_start(wv_t[:], wvs.rearrange("(c p) g -> p c g", p=128))
            for m in range(NP):
                nc.sync.dma_start(wo_t[m][:], wos[m * 128 : (m + 1) * 128, :])
                nc.sync.dma_start(bq_t[m][:], bqs[m])
                nc.sync.dma_start(bk_t[m][:], bks[m])
            nc.sync.dma_start(bv_row[:], bvs[:])
            ones_f = const.tile([128, 128], F32, name="onesf", tag="onesf")
            nc.vector.memset(ones_f[:], 1.0)
            nc.vector.tensor_copy(ones_t[:], ones_f[:])

            # bv broadcast to all partitions: [128, GC] = ones[1,128].T @ bv[1,GC]
            bvb = const.tile([128, GC], F32, name="bvb", tag="bvb")
            bvp = pproj.tile([128, 512], F32, name="pj", tag="pj")
            nc.tensor.matmul(
                bvp[:, :GC], ones_t[0:1, 0:128], bv_row[:],
                start=True, stop=True,
            )
            nc.vector.tensor_copy(bvb[:], bvp[:, :GC])

            # ---- persistent activation tiles ----
            QT = [const.tile([128, S], BF16, name=f"QT{m}", tag=f"QT{m}") for m in range(NP)]
            KT = [const.tile([128, S], BF16, name=f"KT{m}", tag=f"KT{m}") for m in range(NP)]
            # VH: [S-chunk][128, 4 heads, 66] bf16; col 64 = ones, col 65 pad
            VH = [const.tile([128, 4, 66], BF16, name=f"VH{i}", tag=f"VH{i}") for i in range(SKC)]
            ctxT = [const.tile([128, S], BF16, name=f"ctxT{m}", tag=f"ctxT{m}") for m in range(NP)]

            for i in range(SKC):
                nc.vector.memset(VH[i][:, :, 64:65], 1.0)

            # ---- phase 1: projections ----
            # Load order k, v, q: scores need ALL of K but only the first
            # S-chunk of Q, so K/V stream while Q-proj gates attention start.
            def proj_qk(raw, w_t, b_t, dst):
                for nq in range(SQC):
                    for m in range(NP):
                        ps = pproj.tile([128, 512], F32, name="pj", tag="pj")
                        for k in range(KC):
                            nc.tensor.matmul(
                                ps[:],
                                w_t[:, k, m * 128 : (m + 1) * 128],
                                raw[:, k, nq * 512 : (nq + 1) * 512],
                                start=(k == 0),
                                stop=(k == KC - 1),
                            )
                        nc.vector.tensor_scalar_add(
                            dst[m][:, nq * 512 : (nq + 1) * 512], ps[:], b_t[m][:]
                        )

            q_raw = rawp.tile([128, KC, S], BF16, name="qraw", tag="raw", bufs=2)
            nc.sync.dma_start(q_raw[:], qT.rearrange("(c p) s -> p c s", p=128))
            proj_qk(q_raw, wq_t, bq_t, QT)

            k_raw = rawp.tile([128, KC, S], BF16, name="kraw", tag="raw", bufs=2)
            nc.sync.dma_start(k_raw[:], kT.rearrange("(c p) s -> p c s", p=128))
            proj_qk(k_raw, wk_t, bk_t, KT)

            v_raw = rawp.tile([128, KC, S], BF16, name="vraw", tag="raw", bufs=2)
            nc.sync.dma_start(v_raw[:], vT.rearrange("(c p) s -> p c s", p=128))
            for i in range(SKC):  # V in natural [S, GC] layout
                ps = pproj.tile([128, 512], F32, name="pj", tag="pj")
                for k in range(KC):
                    nc.tensor.matmul(
                        ps[:, :GC],
                        v_raw[:, k, i * 128 : (i + 1) * 128],
                        wv_t[:, k, :],
                        start=(k == 0),
                        stop=(k == KC - 1),
                    )
                nc.vector.tensor_add(
                    VH[i][:, :, 0:64],
                    ps[:, :GC].rearrange("p (h d) -> p h d", h=4),
                    bvb[:].rearrange("p (h d) -> p h d", h=4),
                )

            # ---- phase 2 + 3: software-pipelined ----
            # attn_step(sq, m) ends with a fast PSUM evacuation; the slow
            # normalize chain (1-lane reciprocal on DVE) and the dependent
            # replicate/outproj matmuls are deferred one step so the PE's
            # in-order stream never blocks on the DVE chain.
            def attn_step(sq, m, fillers=None):
                fillers = fillers if fillers is not None else []
                ctA = pctx.tile([128, 512], F32, name="ctA", tag="ctA")
                ctB = pctx.tile([128, 512], F32, name="ctB", tag="ctB")
                for i in range(SKC):
                    if i % 2 == 1 and i >= 3 and fillers:
                        fillers.pop(0)()
                    sc = psc.tile([128, 1024], F32, name="sc", tag="sc")
                    nc.tensor.matmul(
                        sc[:, 0:512],
                        KT[m][0:64, i * 128 : (i + 1) * 128],
                        QT[m][0:64, sq * 512 : (sq + 1) * 512],
                        start=True, stop=True,
                    )
                    nc.tensor.matmul(
                        sc[:, 512:1024],
                        KT[m][64:128, i * 128 : (i + 1) * 128],
                        QT[m][64:128, sq * 512 : (sq + 1) * 512],
                        start=True, stop=True,
                        tile_position=(64, 0),
                    )
                    pb = probs.tile([128, 1024], BF16, name="pb", tag="pb")
                    nc.scalar.activation(pb[:], sc[:], EXP, scale=0.125)
                    nc.tensor.matmul(
                        ctA[0:65, :], VH[i][:, 2 * m, 0:65], pb[:, 0:512],
                        start=(i == 0), stop=(i == SKC - 1),
                    )
                    nc.tensor.matmul(
                        ctB[0:65, :], VH[i][:, 2 * m + 1, 0:65], pb[:, 512:1024],
                        start=(i == 0), stop=(i == SKC - 1),
                    )
                # evacuate ctx PSUM quickly so the next pair can start
                ctsb = small.tile([65, 1024], F32, name="ctsb", tag="ctsb")
                nc.vector.tensor_copy(ctsb[0:65, 0:512], ctA[0:65, :])
                nc.vector.tensor_copy(ctsb[0:65, 512:1024], ctB[0:65, :])
                # kick off the reciprocals now (DVE, off critical path)
                r_t = small.tile([128, 1024], F32R, name="rt", tag="rt")
                with nc.allow_low_precision(
                    reason="f32r == fp32 bits; rounding only affects PE reads"
                ):
                    nc.vector.reciprocal(r_t[64:65, 0:512], ctsb[64:65, 0:512])
                    nc.vector.reciprocal(
                        r_t[64:65, 512:1024], ctsb[64:65, 512:1024]
                    )
                return ctsb, r_t

            def norm_step(sq, m, ctsb, r_t):
                rpA = pproj.tile([128, 512], F32, name="pj", tag="pj")
                rpB = pproj.tile([128, 512], F32, name="pj", tag="pj")
                nc.tensor.matmul(
                    rpA[0:64, :], ones_t[64:65, 0:64], r_t[64:65, 0:512],
                    start=True, stop=True, tile_position=(64, 0),
                )
                nc.tensor.matmul(
                    rpB[0:64, :], ones_t[64:65, 0:64], r_t[64:65, 512:1024],
                    start=True, stop=True, tile_position=(64, 0),
                )
                rs = small.tile([64, 1024], F32, name="rs", tag="rs")
                nc.vector.tensor_copy(rs[:, 0:512], rpA[0:64, :])
                nc.vector.tensor_copy(rs[:, 512:1024], rpB[0:64, :])
                nc.vector.tensor_mul(
                    ctxT[m][0:64, sq * 512 : (sq + 1) * 512],
                    ctsb[0:64, 0:512],
                    rs[:, 0:512],
                )
                stgB = small.tile([64, 512], BF16, name="stgB", tag="stgB")
                nc.vector.tensor_mul(
                    stgB[:], ctsb[0:64, 512:1024], rs[:, 512:1024]
                )
                nc.sync.dma_start(
                    ctxT[m][64:128, sq * 512 : (sq + 1) * 512], stgB[:]
                )

            def outproj_group(sq128, ncol):
                def emit():
                    po = pproj.tile([128, 512], F32, name="pj", tag="pj")
                    for m in range(NP):
                        nc.tensor.matmul(
                            po[:],
                            ctxT[m][:, sq128 * 128 : (sq128 + 1) * 128],
                            wo_t[m][:, ncol * 512 : (ncol + 1) * 512],
                            start=(m == 0),
                            stop=(m == NP - 1),
                        )
                    ob = outsb.tile([128, 512], F32, name="ob", tag="ob")
                    nc.vector.tensor_copy(ob[:], po[:])
                    nc.sync.dma_start(
                        outp[
                            sq128 * 128 : (sq128 + 1) * 128,
                            ncol * 512 : (ncol + 1) * 512,
                        ],
                        ob[:],
                    )

                return emit

            def outproj_groups(sq):
                return [
                    outproj_group(sq128, ncol)
                    for sq128 in range(sq * 4, (sq + 1) * 4)
                    for ncol in range(D // 512)
                ]

            # Two-level software pipeline: norm lags attention by one step;
            # outproj groups of a completed sq chunk are interleaved as
            # fillers inside later attention steps so they plug the PE's
            # exp-wait stalls instead of creating their own.
            pending = None
            fill_queue = []
            for step in range(NP * SQC):
                sq, m = step // NP, step % NP
                state = attn_step(sq, m, fill_queue)
                if pending is not None:
                    psq, pm, pctsb, prt = pending
                    norm_step(psq, pm, pctsb, prt)
                    if pm == NP - 1:
                        fill_queue.extend(outproj_groups(psq))
                pending = (sq, m, *state)
            psq, pm, pctsb, prt = pending
            norm_step(psq, pm, pctsb, prt)
            for g in fill_queue + outproj_groups(psq):
                g()

    nc.compile()
    return nc


def get_program():
    if "nc" not in _CACHE:
        _CACHE["nc"] = _build_program()
    return _CACHE["nc"]


def make_in_maps(q, k, v, wq, bq, wk, bk, wv, bv, wo, bo):
    q, k, v = (np.asarray(x, np.float32) for x in (q, k, v))
    wq, wk, wv, wo = (np.asarray(x, np.float32) for x in (wq, wk, wv, wo))
    bq, bk, bv = (np.asarray(x, np.float32) for x in (bq, bk, bv))
    BF = ml_dtypes.bfloat16
    qT = [np.ascontiguousarray(q[b].T).astype(BF) for b in range(B)]
    kTt = [np.ascontiguousarray(k[b].T).astype(BF) for b in range(B)]
    vTt = [np.ascontiguousarray(v[b].T).astype(BF) for b in range(B)]
    in_maps = []
    for c in range(NCORES):
        b, g = c // 4, c % 4
        sl = slice(g * GC, (g + 1) * GC)
        in_maps.append(
            {
                "qT": qT[b],
                "kT": kTt[b],
                "vT": vTt[b],
                "wqs": np.ascontiguousarray(wq[:, sl]).astype(BF),
                "wks": np.ascontiguousarray(wk[:, sl]).astype(BF),
                "wvs": np.ascontiguousarray(wv[:, sl]).astype(BF),
                "wos": np.ascontiguousarray(wo[sl, :]).astype(BF),
                "bqs": np.ascontiguousarray(bq[sl]).reshape(NP, 128, 1),
                "bks": np.ascontiguousarray(bk[sl]).reshape(NP, 128, 1),
                "bvs": np.ascontiguousarray(bv[sl]).reshape(1, GC),
            }
        )
    return in_maps


def combine_outputs(results, bo):
    out = np.zeros((B, S, D), np.float32)
    for c in range(NCORES):
        out[c // 4] += results[c]["outp"]
    out += np.asarray(bo, np.float32)
    return out


def kernel(q, k, v, wq, bq, wk, bk, wv, bv, wo, bo, trace=False):
    from concourse.bass_utils import run_bass_kernel_spmd

    nc = get_program()
    in_maps = make_in_maps(q, k, v, wq, bq, wk, bk, wv, bv, wo, bo)
    res = run_bass_kernel_spmd(nc, in_maps, list(range(NCORES)), trace=trace)
    out = combine_outputs(res.results, bo)
    if trace:
        _CACHE["last_result"] = res
    return out

